# revision 2
# baseline (speedup 1.0000x reference)
"""Causal self-attention (RoPE) Trainium2 Bass kernel, SPMD over 8 NeuronCores.

Sharding: data-parallel over batch (B=2) x tensor-parallel over heads
(16 heads -> 4 heads per core).  core c handles batch c//4, heads
4*(c%4) .. 4*(c%4)+3.  Each core computes its heads' attention output and a
partial out@Wo contribution ([S, H]); the host sums the 4 partials per batch.

Device pipeline per core (q-tile-outer formulation, bf16 matmuls):
  phase1(st): QKV projection (bf16) -> RoPE on DVE/Pool -> x_sb bf16 ->
     tiled DMA-transpose into persistent qT/kT; V (bf16, with an appended
     ones column) kept natural in vo.
  phase2(j):  for each head: transposed score tiles for k-tiles t<=j are
     stacked in PSUM columns -> one exp per <=1024-col stack (ACT) ->
     causal tri mask on the diagonal tile (DVE) -> AV matmuls with the
     P^T tile as STATIONARY and [V | 1] as moving (J=65) accumulating
     O[q, d] + the softmax denominator (col 64) in PSUM.  Normalize via
     reciprocal + broadcast mul, DMA-transpose O^T for phase 3.
  phase3(st): out = O^T @ Wo partial, streamed behind phase 2.
The three phases stream st/j/st-lagged so PE never starves.
"""

import sys
import numpy as np

for _p in ("/opt/trn_rl_repo", "/root/.axon_site/_ro/trn_rl_repo"):
    if _p not in sys.path:
        sys.path.append(_p)

import concourse.bacc as bacc
from concourse import mybir
from concourse.tile import TileContext
from concourse.bass_utils import run_bass_kernel_spmd

F32 = mybir.dt.float32
BF16 = mybir.dt.bfloat16
EXP = mybir.ActivationFunctionType.Exp

NUM_HEADS = 16
HEAD_DIM = 64
ROPE_BASE = 160000.0
N_CORES = 8


def build_nc(S, H, HL):
    """Build the SPMD Bass program.

    S: sequence length; H: hidden size; HL: heads per core (local).
    Per-core tensors: hT [H,S] bf16, wqkv [H, 3*DL] bf16 (cols V|Q|K),
    wo [DL,H] bf16, cs [S,96] f32 (cos|: -sin|: +sin), tri [128,128] bf16
    -> out part [S,H] f32.
    """
    DL = HL * HEAD_DIM          # local channels (256)
    NI = H // 128               # contraction tiles for projections (8)
    NS = S // 128               # sequence tiles (16)
    CT = DL // 128              # channel tiles (2)
    assert DL % 128 == 0 and H % 128 == 0 and S % 128 == 0
    scale = HEAD_DIM ** -0.5

    nc = bacc.Bacc("TRN2", target_bir_lowering=False, debug=False,
                   num_devices=N_CORES)
    hT_d = nc.declare_dram_parameter("hT", [H, S], BF16, isOutput=False)
    wqkv_d = nc.declare_dram_parameter("wqkv", [H, 3 * DL], BF16, isOutput=False)
    wo_d = nc.declare_dram_parameter("wo", [DL, H], BF16, isOutput=False)
    cs_d = nc.declare_dram_parameter("cs", [S, 96], F32, isOutput=False)
    tri_d = nc.declare_dram_parameter("tri", [128, 128], BF16, isOutput=False)
    out_d = nc.declare_dram_parameter("part", [S, H], F32, isOutput=True)

    with TileContext(nc) as tc:
        with (
            tc.tile_pool(name="w", bufs=1) as w_pool,
            tc.tile_pool(name="persist", bufs=1) as pers,
            tc.tile_pool(name="hstream", bufs=3) as hs_pool,
            tc.tile_pool(name="rope", bufs=2) as rope_pool,
            tc.tile_pool(name="xsb", bufs=3) as x_pool,
            tc.tile_pool(name="psb", bufs=4) as p_pool,
            tc.tile_pool(name="norm", bufs=2) as n_pool,
            tc.tile_pool(name="osb", bufs=2) as o_pool,
            tc.tile_pool(name="ps_a", bufs=3, space="PSUM") as ps_a,
            tc.tile_pool(name="ps_o", bufs=2, space="PSUM") as ps_o,
        ):
            # --- weights / constants (resident) ---
            w_t = w_pool.tile([128, NI, 3 * DL], BF16)
            wo_t = w_pool.tile([128, CT, H], BF16)
            cs_t = w_pool.tile([128, NS, 96], F32)
            tri_t = w_pool.tile([128, 128], BF16)
            # weights go down the ACT HWDGE queue (SP streams h tiles);
            # per-i-tile DMAs so the first projection matmul can start as
            # soon as its slice lands.
            wr = wqkv_d[:].rearrange("(t p) d -> p t d", p=128)
            for i in range(NI):
                nc.scalar.dma_start(out=w_t[:, i:i + 1, :], in_=wr[:, i:i + 1, :])
            nc.scalar.dma_start(
                out=wo_t, in_=wo_d[:].rearrange("(t p) o -> p t o", p=128))
            nc.scalar.dma_start(
                out=cs_t, in_=cs_d[:].rearrange("(t p) c -> p t c", p=128))
            nc.scalar.dma_start(out=tri_t, in_=tri_d[:])

            qkT = pers.tile([128, 2, CT, S], BF16)   # [d, q/k, ct, s]
            vo = pers.tile([128, NS, HL, 65], BF16)  # [k, ktile, h, d|1]
            oT = pers.tile([128, CT, S], BF16)       # [dl, ct, s]
            nc.gpsimd.memset(vo[:, :, :, 64:65], 1.0)

            hT_r = hT_d[:].rearrange("(t p) s -> p t s", p=128)
            OV, OQ = 0, DL          # offsets in qkv psum: V | Q,K

            # ---------------- phase 1 ----------------
            def emit_phase1(st):
                s0 = st * 128
                h_t = hs_pool.tile([128, NI, 128], BF16, tag="h")
                nc.sync.dma_start(out=h_t, in_=hT_r[:, :, s0:s0 + 128])

                qkv_ps = ps_a.tile([128, 1024], F32, tag="psa")
                for i in range(NI):
                    nc.tensor.matmul(qkv_ps[:, 0:512], h_t[:, i, :],
                                     w_t[:, i, 0:512],
                                     start=(i == 0), stop=(i == NI - 1))
                    nc.tensor.matmul(qkv_ps[:, 512:768], h_t[:, i, :],
                                     w_t[:, i, 512:768],
                                     start=(i == 0), stop=(i == NI - 1))

                # RoPE directly on the PSUM qkv (cols OQ..OQ+2*DL = q|k)
                cosb = cs_t[:, st, 0:32].unsqueeze(1).broadcast_to(
                    [128, 4 * HL, 32])
                sinmb = cs_t[:, st, 32:64].unsqueeze(1).broadcast_to(
                    [128, 2 * HL, 32])
                sinpb = cs_t[:, st, 64:96].unsqueeze(1).broadcast_to(
                    [128, 2 * HL, 32])
                x_qk = qkv_ps[:, OQ:OQ + 2 * DL]
                x4 = x_qk.rearrange("p (r two d) -> p r two d", two=2, d=32)
                a_t = rope_pool.tile([128, 2 * DL], F32, tag="ra")
                nc.vector.tensor_mul(
                    a_t.rearrange("p (r d) -> p r d", d=32),
                    x_qk.rearrange("p (r d) -> p r d", d=32), cosb)
                b_t = rope_pool.tile([128, 2 * DL], F32, tag="rb")
                b4 = b_t.rearrange("p (r two d) -> p r two d", two=2, d=32)
                nc.vector.tensor_mul(b4[:, :, 0, :], x4[:, :, 1, :], sinmb)
                nc.vector.tensor_mul(b4[:, :, 1, :], x4[:, :, 0, :], sinpb)
                x_sb = x_pool.tile([128, 2 * DL], BF16, tag="x")
                nc.gpsimd.tensor_add(x_sb, a_t, b_t)

                # V copy (Pool): [s, (h d)] -> vo[:, st, h, 0:64] (bf16)
                nc.gpsimd.tensor_copy(
                    vo[:, st, :, 0:64],
                    qkv_ps[:, OV:OV + DL].rearrange("p (h d) -> p h d", d=64))

                # tiled transpose q|k into qkT (one DMA, 4 128-blocks)
                nc.sync.dma_start_transpose(
                    qkT[:, :, :, s0:s0 + 128], x_sb)

            # ---------------- phase 2 ----------------
            def emit_phase2(j):
                o_ps = ps_o.tile([128, 512], F32, tag="o")
                for h in range(HL):
                    base = (h % 2) * 64
                    ct = h // 2
                    qT_h = qkT[base:base + 64, 0, ct, :]
                    kT_h = qkT[base:base + 64, 1, ct, :]
                    oc = h * 65
                    # sub-stacks of <=8 k-tiles (1024 PSUM cols)
                    for t0 in range(0, j + 1, 8):
                        nt = min(8, j + 1 - t0)
                        sc = ps_a.tile([128, 1024], F32, tag="psa")
                        for k in range(nt):
                            t = t0 + k
                            nc.tensor.matmul(
                                sc[:, k * 128:(k + 1) * 128],
                                kT_h[:, t * 128:(t + 1) * 128],
                                qT_h[:, j * 128:(j + 1) * 128],
                                start=True, stop=True)
                        p_sb = p_pool.tile([128, 1024], BF16, tag="p")
                        nc.scalar.activation(p_sb[:, 0:nt * 128],
                                             sc[:, 0:nt * 128], EXP,
                                             scale=scale)
                        if t0 <= j < t0 + nt:  # diagonal tile: causal mask
                            dk = (j - t0) * 128
                            nc.vector.tensor_mul(p_sb[:, dk:dk + 128],
                                                 p_sb[:, dk:dk + 128], tri_t)
                        for k in range(nt):
                            t = t0 + k
                            nc.tensor.matmul(
                                o_ps[:, oc:oc + 65],
                                p_sb[:, k * 128:(k + 1) * 128],
                                vo[:, t, h, :],
                                start=(h == 0 and t == 0),
                                stop=(t == j), skip_group_check=True)

                # normalize all 4 heads: col 64 of each 65 is the denominator
                ov = o_ps[:, 0:HL * 65].rearrange("p (h d) -> p h d", d=65)
                r_sb = n_pool.tile([128, HL], F32, tag="r")
                nc.vector.reciprocal(r_sb, ov[:, :, 64])
                o_sb = n_pool.tile([128, DL], BF16, tag="on")
                nc.vector.tensor_mul(
                    o_sb.rearrange("p (h d) -> p h d", d=64),
                    ov[:, :, 0:64],
                    r_sb.unsqueeze(2).broadcast_to([128, HL, 64]))
                nc.sync.dma_start_transpose(
                    oT[:, :, j * 128:(j + 1) * 128], o_sb)

            # ---------------- phase 3 ----------------
            def emit_phase3(st):
                s0 = st * 128
                wo_ps = ps_a.tile([128, 1024], F32, tag="psa")
                for oc in (0, 512):
                    for ct in range(CT):
                        nc.tensor.matmul(wo_ps[:, oc:oc + 512],
                                         oT[:, ct, s0:s0 + 128],
                                         wo_t[:, ct, oc:oc + 512],
                                         start=(ct == 0), stop=(ct == CT - 1))
                out_sb = o_pool.tile([128, H], F32, tag="out")
                nc.vector.tensor_copy(out_sb[:, 0:512], wo_ps[:, 0:512])
                nc.gpsimd.tensor_copy(out_sb[:, 512:1024], wo_ps[:, 512:1024])
                nc.sync.dma_start(out=out_d[s0:s0 + 128, :], in_=out_sb)

            # ---------------- orchestration ----------------
            LAG = 3
            for st in range(LAG):
                emit_phase1(st)
            for st in range(LAG, NS):
                emit_phase1(st)
                emit_phase2(st - LAG)
                if st - LAG - 1 >= 0:
                    emit_phase3(st - LAG - 1)
            for j in range(NS - LAG, NS):
                emit_phase2(j)
                emit_phase3(j - 1)
            emit_phase3(NS - 1)

    nc.finalize()
    return nc


def rope_tables(S, hd):
    """cos/sin tables matching reference._rope_tables numerics (f32 freqs)."""
    inv = (1.0 / (np.float32(ROPE_BASE) **
                  (np.arange(0, hd, 2, dtype=np.float32) / np.float32(hd))))
    inv = inv.astype(np.float32)
    freqs = (np.arange(S, dtype=np.float32)[:, None] * inv[None, :]
             ).astype(np.float32)
    cos = np.cos(freqs.astype(np.float64)).astype(np.float32)
    sin = np.sin(freqs.astype(np.float64)).astype(np.float32)
    return cos, sin


def _bf16():
    import ml_dtypes
    return ml_dtypes.bfloat16


def make_const_inputs(S):
    """Constant per-core inputs: packed RoPE tables + causal tri mask."""
    bf = _bf16()
    cos, sin = rope_tables(S, HEAD_DIM)
    cs = np.concatenate([cos, -sin, sin], axis=1).astype(np.float32)
    return {
        "cs": np.ascontiguousarray(cs),
        "tri": np.triu(np.ones((128, 128), dtype=np.float32)).astype(bf),
    }


def _is_causal_mask(mask, S):
    m = mask.reshape(S, S)
    rows = np.unique(np.concatenate([np.arange(0, S, max(S // 64, 1)),
                                     [S - 1]]))
    for r in rows:
        row = m[r]
        if not np.all(row[:r + 1] == 0.0):
            return False
        if r + 1 < S and not np.all(row[r + 1:] <= -50.0):
            return False
    return True


_NC_CACHE = {}


def kernel(hidden_states, attention_mask, Wqkv, Wo):
    B, S, H = hidden_states.shape
    nh, hd = NUM_HEADS, HEAD_DIM
    HL = nh // (N_CORES // B)       # heads per core
    DL = HL * hd
    G = N_CORES // B                # cores per batch

    if not _is_causal_mask(np.asarray(attention_mask), S):
        # general-mask fallback: exact host computation
        return _host_reference(hidden_states, attention_mask, Wqkv, Wo)

    key = (S, H, HL)
    if key not in _NC_CACHE:
        _NC_CACHE[key] = build_nc(S, H, HL)
    nc = _NC_CACHE[key]

    bf = _bf16()
    consts = make_const_inputs(S)

    hs = np.asarray(hidden_states, dtype=np.float32)
    Wqkv = np.asarray(Wqkv, dtype=np.float32)
    Wo = np.asarray(Wo, dtype=np.float32)
    hT = [np.ascontiguousarray(hs[b].T).astype(bf) for b in range(B)]

    in_maps = []
    for c in range(N_CORES):
        b, g = divmod(c, G)
        c0 = g * DL
        wqkv = np.concatenate([
            Wqkv[:, 2 * H + c0:2 * H + c0 + DL],   # V
            Wqkv[:, c0:c0 + DL],                   # Q
            Wqkv[:, H + c0:H + c0 + DL],           # K
        ], axis=1).astype(bf)
        in_maps.append({
            "hT": hT[b],
            "wqkv": np.ascontiguousarray(wqkv),
            "wo": np.ascontiguousarray(Wo[c0:c0 + DL, :].astype(bf)),
            **consts,
        })

    res = run_bass_kernel_spmd(nc, in_maps, list(range(N_CORES)))
    out = np.empty((B, S, H), dtype=np.float32)
    for b in range(B):
        acc = res.results[b * G]["part"].astype(np.float64)
        for g in range(1, G):
            acc += res.results[b * G + g]["part"]
        out[b] = acc.astype(np.float32)
    return out


def _host_reference(hidden_states, attention_mask, Wqkv, Wo):
    """Exact fallback for non-causal masks (numpy, fp32)."""
    B, S, H = hidden_states.shape
    nh, hd = NUM_HEADS, HEAD_DIM
    cos, sin = rope_tables(S, hd)
    qkv = hidden_states.reshape(B * S, H) @ Wqkv
    qkv = qkv.reshape(B, S, 3, nh, hd).transpose(2, 0, 3, 1, 4)
    q, k, v = qkv[0], qkv[1], qkv[2]

    def rope(x):
        x1, x2 = x[..., :hd // 2], x[..., hd // 2:]
        c, s = cos[None, None], sin[None, None]
        return np.concatenate([x1 * c - x2 * s, x2 * c + x1 * s], axis=-1)

    q, k = rope(q), rope(k)
    scores = np.einsum('bhqd,bhkd->bhqk', q, k) * (hd ** -0.5)
    scores = scores + attention_mask.reshape(1, 1, S, S)
    scores -= scores.max(axis=-1, keepdims=True)
    e = np.exp(scores)
    attn = e / e.sum(axis=-1, keepdims=True)
    out = np.einsum('bhqk,bhkd->bhqd', attn, v)
    out = out.transpose(0, 2, 1, 3).reshape(B, S, H)
    return (out @ Wo).astype(np.float32)


# revision 5
# speedup vs baseline: 1.0151x; 1.0151x over previous
"""Causal self-attention (RoPE) Trainium2 Bass kernel, SPMD over 8 NeuronCores.

Sharding: data-parallel over batch (B=2) x tensor-parallel over heads
(16 heads -> 4 heads per core).  core c handles batch c//4, heads
4*(c%4) .. 4*(c%4)+3.  Each core computes its heads' attention output and a
partial out@Wo contribution ([S, H]); the host sums the 4 partials per batch.

Device pipeline per core (q-tile-outer formulation, bf16 matmuls):
  phase1(st): QKV projection (bf16) -> RoPE on DVE/Pool -> x_sb bf16 ->
     tiled DMA-transpose into persistent qT/kT; V (bf16, with an appended
     ones column) kept natural in vo.
  phase2(j):  for each head: transposed score tiles for k-tiles t<=j are
     stacked in PSUM columns -> one exp per <=1024-col stack (ACT) ->
     causal tri mask on the diagonal tile (DVE) -> AV matmuls with the
     P^T tile as STATIONARY and [V | 1] as moving (J=65) accumulating
     O[q, d] + the softmax denominator (col 64) in PSUM.  Normalize via
     reciprocal + broadcast mul, DMA-transpose O^T for phase 3.
  phase3(st): out = O^T @ Wo partial, streamed behind phase 2.
The three phases stream st/j/st-lagged so PE never starves.
"""

import sys
import numpy as np

for _p in ("/opt/trn_rl_repo", "/root/.axon_site/_ro/trn_rl_repo"):
    if _p not in sys.path:
        sys.path.append(_p)

import concourse.bacc as bacc
from concourse import mybir
from concourse.tile import TileContext
from concourse.bass_utils import run_bass_kernel_spmd

F32 = mybir.dt.float32
BF16 = mybir.dt.bfloat16
EXP = mybir.ActivationFunctionType.Exp

NUM_HEADS = 16
HEAD_DIM = 64
ROPE_BASE = 160000.0
N_CORES = 8


def build_nc(S, H, HL):
    """Build the SPMD Bass program.

    S: sequence length; H: hidden size; HL: heads per core (local).
    Per-core tensors: hT [H,S] bf16, wqkv [H, 3*DL] bf16 (cols V|Q|K),
    wo [DL,H] bf16, cs [S,96] f32 (cos|: -sin|: +sin), tri [128,128] bf16
    -> out part [S,H] f32.
    """
    DL = HL * HEAD_DIM          # local channels (256)
    NI = H // 128               # contraction tiles for projections (8)
    NS = S // 128               # sequence tiles (16)
    CT = DL // 128              # channel tiles (2)
    assert DL % 128 == 0 and H % 128 == 0 and S % 128 == 0
    scale = HEAD_DIM ** -0.5

    nc = bacc.Bacc("TRN2", target_bir_lowering=False, debug=False,
                   num_devices=N_CORES)
    hT_d = nc.declare_dram_parameter("hT", [H, S], BF16, isOutput=False)
    wqkv_d = nc.declare_dram_parameter("wqkv", [H, 3 * DL], BF16, isOutput=False)
    wo_d = nc.declare_dram_parameter("wo", [DL, H], BF16, isOutput=False)
    cs_d = nc.declare_dram_parameter("cs", [S, 96], F32, isOutput=False)
    tri_d = nc.declare_dram_parameter("tri", [128, 128], BF16, isOutput=False)
    out_d = nc.declare_dram_parameter("part", [S, H], F32, isOutput=True)

    with TileContext(nc) as tc:
        with (
            tc.tile_pool(name="w", bufs=1) as w_pool,
            tc.tile_pool(name="persist", bufs=1) as pers,
            tc.tile_pool(name="hstream", bufs=3) as hs_pool,
            tc.tile_pool(name="rope", bufs=2) as rope_pool,
            tc.tile_pool(name="xsb", bufs=3) as x_pool,
            tc.tile_pool(name="psb", bufs=4) as p_pool,
            tc.tile_pool(name="norm", bufs=2) as n_pool,
            tc.tile_pool(name="osb", bufs=2) as o_pool,
            tc.tile_pool(name="ps_a", bufs=3, space="PSUM") as ps_a,
            tc.tile_pool(name="ps_o", bufs=2, space="PSUM") as ps_o,
        ):
            # --- weights / constants (resident) ---
            w_t = w_pool.tile([128, NI, 3 * DL], BF16)
            wo_t = w_pool.tile([128, CT, H], BF16)
            cs_t = w_pool.tile([128, NS, 96], F32)
            tri_t = w_pool.tile([128, 128], BF16)
            # weights go down the ACT HWDGE queue (SP streams h tiles);
            # per-i-tile DMAs so the first projection matmul can start as
            # soon as its slice lands.
            wr = wqkv_d[:].rearrange("(t p) d -> p t d", p=128)
            for i in range(NI):
                nc.scalar.dma_start(out=w_t[:, i:i + 1, :], in_=wr[:, i:i + 1, :])
            nc.scalar.dma_start(
                out=wo_t, in_=wo_d[:].rearrange("(t p) o -> p t o", p=128))
            nc.scalar.dma_start(
                out=cs_t, in_=cs_d[:].rearrange("(t p) c -> p t c", p=128))
            nc.scalar.dma_start(out=tri_t, in_=tri_d[:])

            qkT = pers.tile([128, 2, CT, S], BF16)   # [d, q/k, ct, s]
            vo = pers.tile([128, NS, HL, 65], BF16)  # [k, ktile, h, d|1]
            oT = pers.tile([128, CT, S], BF16)       # [dl, ct, s]
            nc.gpsimd.memset(vo[:, :, :, 64:65], 1.0)

            hT_r = hT_d[:].rearrange("(t p) s -> p t s", p=128)
            OV, OQ = 0, DL          # offsets in qkv psum: V | Q,K

            # ---------------- phase 1 ----------------
            def emit_phase1(st):
                s0 = st * 128
                h_t = hs_pool.tile([128, NI, 128], BF16, tag="h")
                nc.sync.dma_start(out=h_t, in_=hT_r[:, :, s0:s0 + 128])

                qkv_ps = ps_a.tile([128, 1024], F32, tag="psa")
                for i in range(NI):
                    nc.tensor.matmul(qkv_ps[:, 0:512], h_t[:, i, :],
                                     w_t[:, i, 0:512],
                                     start=(i == 0), stop=(i == NI - 1))
                    nc.tensor.matmul(qkv_ps[:, 512:768], h_t[:, i, :],
                                     w_t[:, i, 512:768],
                                     start=(i == 0), stop=(i == NI - 1))

                # RoPE directly on the PSUM qkv (cols OQ..OQ+2*DL = q|k)
                cosb = cs_t[:, st, 0:32].unsqueeze(1).broadcast_to(
                    [128, 4 * HL, 32])
                sinmb = cs_t[:, st, 32:64].unsqueeze(1).broadcast_to(
                    [128, 2 * HL, 32])
                sinpb = cs_t[:, st, 64:96].unsqueeze(1).broadcast_to(
                    [128, 2 * HL, 32])
                x_qk = qkv_ps[:, OQ:OQ + 2 * DL]
                x4 = x_qk.rearrange("p (r two d) -> p r two d", two=2, d=32)
                a_t = rope_pool.tile([128, 2 * DL], F32, tag="ra")
                nc.vector.tensor_mul(
                    a_t.rearrange("p (r d) -> p r d", d=32),
                    x_qk.rearrange("p (r d) -> p r d", d=32), cosb)
                b_t = rope_pool.tile([128, 2 * DL], F32, tag="rb")
                b4 = b_t.rearrange("p (r two d) -> p r two d", two=2, d=32)
                nc.vector.tensor_mul(b4[:, :, 0, :], x4[:, :, 1, :], sinmb)
                nc.vector.tensor_mul(b4[:, :, 1, :], x4[:, :, 0, :], sinpb)
                x_sb = x_pool.tile([128, 2 * DL], BF16, tag="x")
                nc.gpsimd.tensor_add(x_sb, a_t, b_t)

                # V copy: [s, (h d)] -> vo[:, st, h, 0:64] (bf16).  DVE --
                # gpsimd cannot read PSUM.
                nc.vector.tensor_copy(
                    vo[:, st, :, 0:64],
                    qkv_ps[:, OV:OV + DL].rearrange("p (h d) -> p h d", d=64))

                # tiled transpose q|k into qkT (one DMA, 4 128-blocks)
                nc.sync.dma_start_transpose(
                    qkT[:, :, :, s0:s0 + 128], x_sb)

            # ---------------- phase 2 ----------------
            # AV matmuls are deferred two sub-stacks behind scores/exp so
            # the PE never waits on the ACT exp: the engine executes its
            # stream in order, so an AV emitted right after its exp stalls
            # the PE for the exp's full latency.
            avq = []
            o_live = {}

            def flush_avq(keep=0):
                while len(avq) > keep:
                    avq.pop(0)()

            def emit_phase2_scores(j):
                o_ps = ps_o.tile([128, 512], F32, tag="o")
                o_live[j] = o_ps
                for h in range(HL):
                    base = (h % 2) * 64
                    ct = h // 2
                    qT_h = qkT[base:base + 64, 0, ct, :]
                    kT_h = qkT[base:base + 64, 1, ct, :]
                    oc = h * 65
                    # sub-stacks of <=8 k-tiles (1024 PSUM cols)
                    for t0 in range(0, j + 1, 8):
                        nt = min(8, j + 1 - t0)
                        sc = ps_a.tile([128, 1024], F32, tag="psa")
                        for k in range(nt):
                            t = t0 + k
                            nc.tensor.matmul(
                                sc[:, k * 128:(k + 1) * 128],
                                kT_h[:, t * 128:(t + 1) * 128],
                                qT_h[:, j * 128:(j + 1) * 128],
                                start=True, stop=True)
                        flush_avq(keep=2)
                        p_sb = p_pool.tile([128, 1024], BF16, tag="p")
                        nc.scalar.activation(p_sb[:, 0:nt * 128],
                                             sc[:, 0:nt * 128], EXP,
                                             scale=scale)
                        if t0 <= j < t0 + nt:  # diagonal tile: causal mask
                            dk = (j - t0) * 128
                            nc.vector.tensor_mul(p_sb[:, dk:dk + 128],
                                                 p_sb[:, dk:dk + 128], tri_t)

                        def av(h=h, t0=t0, nt=nt, oc=oc, p_sb=p_sb):
                            for k in range(nt):
                                t = t0 + k
                                nc.tensor.matmul(
                                    o_ps[:, oc:oc + 65],
                                    p_sb[:, k * 128:(k + 1) * 128],
                                    vo[:, t, h, :],
                                    start=(h == 0 and t == 0),
                                    stop=(t == j), skip_group_check=True)
                        avq.append(av)

            def emit_phase2_norm(j):
                flush_avq(0)
                o_ps = o_live.pop(j)
                # normalize: col 64 of each head's 65 is the denominator
                ov = o_ps[:, 0:HL * 65].rearrange("p (h d) -> p h d", d=65)
                r_sb = n_pool.tile([128, HL], F32, tag="r")
                nc.vector.reciprocal(r_sb, ov[:, :, 64])
                o_sb = n_pool.tile([128, DL], BF16, tag="on")
                nc.vector.tensor_mul(
                    o_sb.rearrange("p (h d) -> p h d", d=64),
                    ov[:, :, 0:64],
                    r_sb.unsqueeze(2).broadcast_to([128, HL, 64]))
                nc.sync.dma_start_transpose(
                    oT[:, :, j * 128:(j + 1) * 128], o_sb)

            # ---------------- phase 3 ----------------
            def emit_phase3(st):
                s0 = st * 128
                wo_ps = ps_a.tile([128, 1024], F32, tag="psa")
                for oc in (0, 512):
                    for ct in range(CT):
                        nc.tensor.matmul(wo_ps[:, oc:oc + 512],
                                         oT[:, ct, s0:s0 + 128],
                                         wo_t[:, ct, oc:oc + 512],
                                         start=(ct == 0), stop=(ct == CT - 1))
                out_sb = o_pool.tile([128, H], F32, tag="out")
                nc.vector.tensor_copy(out_sb[:, 0:512], wo_ps[:, 0:512])
                nc.scalar.copy(out_sb[:, 512:1024], wo_ps[:, 512:1024])
                nc.sync.dma_start(out=out_d[s0:s0 + 128, :], in_=out_sb)

            # ---------------- orchestration ----------------
            # phase3 matmuls sit between the last scores/exp of phase2(j)
            # and the final AV flush, so the PE has guaranteed work while
            # the last exp completes.
            LAG = 3
            for st in range(LAG):
                emit_phase1(st)
            for st in range(LAG, NS):
                emit_phase1(st)
                emit_phase2_scores(st - LAG)
                if st - LAG - 1 >= 0:
                    emit_phase3(st - LAG - 1)
                emit_phase2_norm(st - LAG)
            for j in range(NS - LAG, NS):
                emit_phase2_scores(j)
                emit_phase3(j - 1)
                emit_phase2_norm(j)
            emit_phase3(NS - 1)

    nc.finalize()
    return nc


def rope_tables(S, hd):
    """cos/sin tables matching reference._rope_tables numerics (f32 freqs)."""
    inv = (1.0 / (np.float32(ROPE_BASE) **
                  (np.arange(0, hd, 2, dtype=np.float32) / np.float32(hd))))
    inv = inv.astype(np.float32)
    freqs = (np.arange(S, dtype=np.float32)[:, None] * inv[None, :]
             ).astype(np.float32)
    cos = np.cos(freqs.astype(np.float64)).astype(np.float32)
    sin = np.sin(freqs.astype(np.float64)).astype(np.float32)
    return cos, sin


def _bf16():
    import ml_dtypes
    return ml_dtypes.bfloat16


def make_const_inputs(S):
    """Constant per-core inputs: packed RoPE tables + causal tri mask."""
    bf = _bf16()
    cos, sin = rope_tables(S, HEAD_DIM)
    cs = np.concatenate([cos, -sin, sin], axis=1).astype(np.float32)
    return {
        "cs": np.ascontiguousarray(cs),
        "tri": np.triu(np.ones((128, 128), dtype=np.float32)).astype(bf),
    }


def _is_causal_mask(mask, S):
    m = mask.reshape(S, S)
    rows = np.unique(np.concatenate([np.arange(0, S, max(S // 64, 1)),
                                     [S - 1]]))
    for r in rows:
        row = m[r]
        if not np.all(row[:r + 1] == 0.0):
            return False
        if r + 1 < S and not np.all(row[r + 1:] <= -50.0):
            return False
    return True


_NC_CACHE = {}


def kernel(hidden_states, attention_mask, Wqkv, Wo):
    B, S, H = hidden_states.shape
    nh, hd = NUM_HEADS, HEAD_DIM
    HL = nh // (N_CORES // B)       # heads per core
    DL = HL * hd
    G = N_CORES // B                # cores per batch

    if not _is_causal_mask(np.asarray(attention_mask), S):
        # general-mask fallback: exact host computation
        return _host_reference(hidden_states, attention_mask, Wqkv, Wo)

    key = (S, H, HL)
    if key not in _NC_CACHE:
        _NC_CACHE[key] = build_nc(S, H, HL)
    nc = _NC_CACHE[key]

    bf = _bf16()
    consts = make_const_inputs(S)

    hs = np.asarray(hidden_states, dtype=np.float32)
    Wqkv = np.asarray(Wqkv, dtype=np.float32)
    Wo = np.asarray(Wo, dtype=np.float32)
    hT = [np.ascontiguousarray(hs[b].T).astype(bf) for b in range(B)]

    in_maps = []
    for c in range(N_CORES):
        b, g = divmod(c, G)
        c0 = g * DL
        wqkv = np.concatenate([
            Wqkv[:, 2 * H + c0:2 * H + c0 + DL],   # V
            Wqkv[:, c0:c0 + DL],                   # Q
            Wqkv[:, H + c0:H + c0 + DL],           # K
        ], axis=1).astype(bf)
        in_maps.append({
            "hT": hT[b],
            "wqkv": np.ascontiguousarray(wqkv),
            "wo": np.ascontiguousarray(Wo[c0:c0 + DL, :].astype(bf)),
            **consts,
        })

    res = run_bass_kernel_spmd(nc, in_maps, list(range(N_CORES)))
    out = np.empty((B, S, H), dtype=np.float32)
    for b in range(B):
        acc = res.results[b * G]["part"].astype(np.float64)
        for g in range(1, G):
            acc += res.results[b * G + g]["part"]
        out[b] = acc.astype(np.float32)
    return out


def _host_reference(hidden_states, attention_mask, Wqkv, Wo):
    """Exact fallback for non-causal masks (numpy, fp32)."""
    B, S, H = hidden_states.shape
    nh, hd = NUM_HEADS, HEAD_DIM
    cos, sin = rope_tables(S, hd)
    qkv = hidden_states.reshape(B * S, H) @ Wqkv
    qkv = qkv.reshape(B, S, 3, nh, hd).transpose(2, 0, 3, 1, 4)
    q, k, v = qkv[0], qkv[1], qkv[2]

    def rope(x):
        x1, x2 = x[..., :hd // 2], x[..., hd // 2:]
        c, s = cos[None, None], sin[None, None]
        return np.concatenate([x1 * c - x2 * s, x2 * c + x1 * s], axis=-1)

    q, k = rope(q), rope(k)
    scores = np.einsum('bhqd,bhkd->bhqk', q, k) * (hd ** -0.5)
    scores = scores + attention_mask.reshape(1, 1, S, S)
    scores -= scores.max(axis=-1, keepdims=True)
    e = np.exp(scores)
    attn = e / e.sum(axis=-1, keepdims=True)
    out = np.einsum('bhqk,bhkd->bhqd', attn, v)
    out = out.transpose(0, 2, 1, 3).reshape(B, S, H)
    return (out @ Wo).astype(np.float32)


# revision 28
# speedup vs baseline: 1.1917x; 1.1740x over previous
"""Causal self-attention (RoPE) Trainium2 Bass kernel, SPMD over 8 NeuronCores.

Sharding: data-parallel over batch (B=2) x tensor-parallel over heads
(16 heads -> 4 heads per core).  core c handles batch c//4, heads
4*(c%4) .. 4*(c%4)+3.  Each core computes its heads' attention output and a
partial out@Wo contribution ([S, H]); the host sums the 4 partials per batch.

Device pipeline per core (q-tile-outer formulation, bf16 matmuls):
  phase1(st): QKV projection (bf16) -> RoPE on DVE/Pool -> x_sb bf16 ->
     tiled DMA-transpose into persistent qT/kT; V (bf16, with an appended
     ones column) kept natural in vo.
  phase2(j):  for each head: transposed score tiles for k-tiles t<=j are
     stacked in PSUM columns -> one exp per <=1024-col stack (ACT) ->
     causal tri mask on the diagonal tile (DVE) -> AV matmuls with the
     P^T tile as STATIONARY and [V | 1] as moving (J=65) accumulating
     O[q, d] + the softmax denominator (col 64) in PSUM.  Normalize via
     reciprocal + broadcast mul, DMA-transpose O^T for phase 3.
  phase3(st): out = O^T @ Wo partial, streamed behind phase 2.
The three phases stream st/j/st-lagged so PE never starves.
"""

import sys
import numpy as np

for _p in ("/opt/trn_rl_repo", "/root/.axon_site/_ro/trn_rl_repo"):
    if _p not in sys.path:
        sys.path.append(_p)

import concourse.bacc as bacc
from concourse import mybir
from concourse.tile import TileContext
from concourse.bass_utils import run_bass_kernel_spmd

F32 = mybir.dt.float32
BF16 = mybir.dt.bfloat16
EXP = mybir.ActivationFunctionType.Exp

NUM_HEADS = 16
HEAD_DIM = 64
ROPE_BASE = 160000.0
N_CORES = 8


def build_nc(S, H, HL):
    """Build the SPMD Bass program.

    S: sequence length; H: hidden size; HL: heads per core (local).
    Per-core tensors: hT [H,S] bf16, wqkv [H, 3*DL] bf16 (cols V|Q|K),
    wo [DL,H] bf16, cs [S,96] f32 (cos|: -sin|: +sin), tri [128,128] bf16
    -> out part [S,H] f32.
    """
    DL = HL * HEAD_DIM          # local channels (256)
    NI = H // 128               # contraction tiles for projections (8)
    NS = S // 128               # sequence tiles (16)
    CT = DL // 128              # channel tiles (2)
    assert DL % 128 == 0 and H % 128 == 0 and S % 128 == 0
    scale = HEAD_DIM ** -0.5

    nc = bacc.Bacc("TRN2", target_bir_lowering=False, debug=False,
                   num_devices=N_CORES)
    hT_d = nc.declare_dram_parameter("hT", [H, S], BF16, isOutput=False)
    wqkv_d = nc.declare_dram_parameter("wqkv", [H, 3 * DL], BF16, isOutput=False)
    wo_d = nc.declare_dram_parameter("wo", [DL, H], BF16, isOutput=False)
    cs_d = nc.declare_dram_parameter("cs", [S, 96], F32, isOutput=False)
    tri_d = nc.declare_dram_parameter("tri", [128, 128], BF16, isOutput=False)
    id_d = nc.declare_dram_parameter("ident", [128, 128], BF16, isOutput=False)
    out_d = nc.declare_dram_parameter("part", [S, H], F32, isOutput=True)

    with TileContext(nc) as tc:
        with (
            tc.tile_pool(name="w", bufs=1) as w_pool,
            tc.tile_pool(name="persist", bufs=1) as pers,
            tc.tile_pool(name="hstream", bufs=5) as hs_pool,
            tc.tile_pool(name="rope", bufs=2) as rope_pool,
            tc.tile_pool(name="xsb", bufs=4) as x_pool,
            tc.tile_pool(name="psb", bufs=4) as p_pool,
            tc.tile_pool(name="norm", bufs=4) as n_pool,
            tc.tile_pool(name="osb", bufs=3) as o_pool,
            tc.tile_pool(name="ps_a", bufs=3, space="PSUM") as ps_a,
            tc.tile_pool(name="ps_o", bufs=2, space="PSUM") as ps_o,
        ):
            # --- weights / constants (resident) ---
            w_t = w_pool.tile([128, NI, 3 * DL], BF16)
            wo_t = w_pool.tile([128, CT, H], BF16)
            cs_t = w_pool.tile([128, NS, 96], F32)
            tri_t = w_pool.tile([128, 128], BF16)
            id_t = w_pool.tile([128, 128], BF16)
            # weights go down the ACT HWDGE queue (SP streams h tiles);
            # per-i-tile DMAs so the first projection matmul can start as
            # soon as its slice lands.
            wr = wqkv_d[:].rearrange("(t p) d -> p t d", p=128)
            for i in range(NI):
                nc.scalar.dma_start(out=w_t[:, i:i + 1, :], in_=wr[:, i:i + 1, :])
            nc.scalar.dma_start(
                out=wo_t, in_=wo_d[:].rearrange("(t p) o -> p t o", p=128))
            nc.scalar.dma_start(
                out=cs_t, in_=cs_d[:].rearrange("(t p) c -> p t c", p=128))
            nc.scalar.dma_start(out=tri_t, in_=tri_d[:])
            nc.scalar.dma_start(out=id_t, in_=id_d[:])

            qkT = pers.tile([128, 2, CT, S], BF16)   # [d, q/k, ct, s]
            vo = pers.tile([128, NS, HL, 65], BF16)  # [k, ktile, h, d|1]
            oT = pers.tile([128, CT, S], BF16)       # [dl, ct, s]
            nc.gpsimd.memset(vo[:, :, :, 64:65], 1.0)

            hT_r = hT_d[:].rearrange("(t p) s -> p t s", p=128)
            OV, OQ = 0, DL          # offsets in qkv psum: V | Q,K

            # DMA deferral: a HWDGE sequencer processes DMAs in order and
            # BLOCKS while a DMA's source isn't ready, so every DMA whose
            # source is produced in iteration i is dispatched in iteration
            # i+2 (source long done -> the queue never stalls).  Producer
            # DMAs (h loads, qkT) go down SP; consumer-side stores (oT,
            # out) go down the ACT queue, which is otherwise idle between
            # exps.
            spq, spq_next, actq, actq_next = [], [], [], []

            def flush_dma_queues():
                while spq:
                    spq.pop(0)()
                while actq:
                    actq.pop(0)()
                spq.extend(spq_next)
                actq.extend(actq_next)
                del spq_next[:], actq_next[:]

            # ---------------- phase 1 ----------------
            def emit_phase1(st):
                s0 = st * 128
                h_t = hs_pool.tile([128, NI, 128], BF16, tag="h")
                nc.sync.dma_start(out=h_t, in_=hT_r[:, :, s0:s0 + 128])
                flush_dma_queues()

                qkv_ps = ps_a.tile([128, 1024], F32, tag="psa")
                for i in range(NI):
                    nc.tensor.matmul(qkv_ps[:, 0:512], h_t[:, i, :],
                                     w_t[:, i, 0:512],
                                     start=(i == 0), stop=(i == NI - 1))
                    nc.tensor.matmul(qkv_ps[:, 512:768], h_t[:, i, :],
                                     w_t[:, i, 512:768],
                                     start=(i == 0), stop=(i == NI - 1))

                # V copy first: AV matmuls need vo sooner than qkT's
                # consumers need the rope output.  DVE -- gpsimd cannot
                # read PSUM.
                nc.vector.tensor_copy(
                    vo[:, st, :, 0:64],
                    qkv_ps[:, OV:OV + DL].rearrange("p (h d) -> p h d", d=64))

                # RoPE directly on the PSUM qkv (cols OQ..OQ+2*DL = q|k)
                cosb = cs_t[:, st, 0:32].unsqueeze(1).broadcast_to(
                    [128, 4 * HL, 32])
                sinmb = cs_t[:, st, 32:64].unsqueeze(1).broadcast_to(
                    [128, 2 * HL, 32])
                sinpb = cs_t[:, st, 64:96].unsqueeze(1).broadcast_to(
                    [128, 2 * HL, 32])
                x_qk = qkv_ps[:, OQ:OQ + 2 * DL]
                x4 = x_qk.rearrange("p (r two d) -> p r two d", two=2, d=32)
                a_t = rope_pool.tile([128, 2 * DL], F32, tag="ra")
                nc.vector.tensor_mul(
                    a_t.rearrange("p (r d) -> p r d", d=32),
                    x_qk.rearrange("p (r d) -> p r d", d=32), cosb)
                b_t = rope_pool.tile([128, 2 * DL], F32, tag="rb")
                b4 = b_t.rearrange("p (r two d) -> p r two d", two=2, d=32)
                nc.vector.tensor_mul(b4[:, :, 0, :], x4[:, :, 1, :], sinmb)
                nc.vector.tensor_mul(b4[:, :, 1, :], x4[:, :, 0, :], sinpb)
                x_sb = x_pool.tile([128, 2 * DL], BF16, tag="x")
                nc.gpsimd.tensor_add(x_sb, a_t, b_t)

                # tiled transpose q|k into qkT (one DMA, 4 128-blocks)
                spq_next.append(
                    lambda x_sb=x_sb, s0=s0: nc.sync.dma_start_transpose(
                        qkT[:, :, :, s0:s0 + 128], x_sb))

            # ---------------- phase 2 ----------------
            # AV matmuls are deferred two sub-stacks behind scores/exp so
            # the PE never waits on the ACT exp: the engine executes its
            # stream in order, so an AV emitted right after its exp stalls
            # the PE for the exp's full latency.
            avq = []
            peq = []   # deferred O-transpose closures (PE + DVE copy)
            o_live = {}

            def flush_avq(keep=0):
                while len(avq) > keep:
                    avq.pop(0)()

            def flush_peq():
                while peq:
                    peq.pop(0)()

            def emit_phase2_scores(j):
                flush_peq()
                o_ps = ps_o.tile([128, 512], F32, tag="o")
                o_live[j] = o_ps
                for h in range(HL):
                    base = (h % 2) * 64
                    ct = h // 2
                    qT_h = qkT[base:base + 64, 0, ct, :]
                    kT_h = qkT[base:base + 64, 1, ct, :]
                    oc = h * 65
                    # sub-stacks of <=8 k-tiles (1024 PSUM cols)
                    for t0 in range(0, j + 1, 8):
                        nt = min(8, j + 1 - t0)
                        sc = ps_a.tile([128, 1024], F32, tag="psa")
                        for k in range(nt):
                            t = t0 + k
                            nc.tensor.matmul(
                                sc[:, k * 128:(k + 1) * 128],
                                kT_h[:, t * 128:(t + 1) * 128],
                                qT_h[:, j * 128:(j + 1) * 128],
                                start=True, stop=True)
                        flush_avq(keep=2)
                        p_sb = p_pool.tile([128, 1024], BF16, tag="p")
                        nc.scalar.activation(p_sb[:, 0:nt * 128],
                                             sc[:, 0:nt * 128], EXP,
                                             scale=scale)
                        if t0 <= j < t0 + nt:  # diagonal tile: causal mask
                            dk = (j - t0) * 128
                            nc.vector.tensor_mul(p_sb[:, dk:dk + 128],
                                                 p_sb[:, dk:dk + 128], tri_t)

                        def av(h=h, t0=t0, nt=nt, oc=oc, p_sb=p_sb):
                            for k in range(nt):
                                t = t0 + k
                                nc.tensor.matmul(
                                    o_ps[:, oc:oc + 65],
                                    p_sb[:, k * 128:(k + 1) * 128],
                                    vo[:, t, h, :],
                                    start=(h == 0 and t == 0),
                                    stop=(t == j), skip_group_check=True)
                        avq.append(av)

            def emit_phase2_norm(j):
                flush_avq(0)
                o_ps = o_live.pop(j)
                # normalize: col 64 of each head's 65 is the denominator
                ov = o_ps[:, 0:HL * 65].rearrange("p (h d) -> p h d", d=65)
                r_sb = n_pool.tile([128, HL], F32, tag="r")
                nc.vector.reciprocal(r_sb, ov[:, :, 64])
                o_sb = n_pool.tile([128, DL], BF16, tag="on")
                nc.vector.tensor_mul(
                    o_sb.rearrange("p (h d) -> p h d", d=64),
                    ov[:, :, 0:64],
                    r_sb.unsqueeze(2).broadcast_to([128, HL, 64]))

                # O^T on the PE (bf16 transpose, 53ns/tile) into the spare
                # half of the o_ps bank, then one DVE copy into oT.
                # Deferred one iteration so the PE never waits on the norm.
                def otrans(j=j, o_ps=o_ps, o_sb=o_sb):
                    tsl = o_ps[:, 256:384].bitcast(BF16)
                    for ct in range(CT):
                        nc.tensor.transpose(
                            tsl[:, ct * 128:(ct + 1) * 128],
                            o_sb[:, ct * 128:(ct + 1) * 128], id_t)
                    nc.vector.tensor_copy(
                        oT[:, :, j * 128:(j + 1) * 128],
                        tsl.rearrange("p (c s) -> p c s", s=128))
                peq.append(otrans)

            # ---------------- phase 3 ----------------
            def emit_phase3(st):
                s0 = st * 128
                wo_ps = ps_a.tile([128, 1024], F32, tag="psa")
                for oc in (0, 512):
                    for ct in range(CT):
                        nc.tensor.matmul(wo_ps[:, oc:oc + 512],
                                         oT[:, ct, s0:s0 + 128],
                                         wo_t[:, ct, oc:oc + 512],
                                         start=(ct == 0), stop=(ct == CT - 1))
                out_sb = o_pool.tile([128, H], F32, tag="out")
                nc.vector.tensor_copy(out_sb[:, 0:512], wo_ps[:, 0:512])
                nc.scalar.copy(out_sb[:, 512:1024], wo_ps[:, 512:1024])
                actq_next.append(lambda s0=s0, out_sb=out_sb: nc.scalar.dma_start(
                    out=out_d[s0:s0 + 128, :], in_=out_sb))

            # ---------------- orchestration ----------------
            # phase3 matmuls sit between the last scores/exp of phase2(j)
            # and the final AV flush, so the PE has guaranteed work while
            # the last exp completes.
            LAG = 3
            for st in range(LAG):
                emit_phase1(st)
            for st in range(LAG, NS):
                emit_phase1(st)
                emit_phase2_scores(st - LAG)
                if st - LAG - 1 >= 0:
                    emit_phase3(st - LAG - 1)
                emit_phase2_norm(st - LAG)
            for j in range(NS - LAG, NS):
                flush_dma_queues()
                emit_phase2_scores(j)
                emit_phase3(j - 1)
                emit_phase2_norm(j)
            flush_peq()
            emit_phase3(NS - 1)
            flush_dma_queues()
            flush_dma_queues()

    nc.finalize()
    return nc


def rope_tables(S, hd):
    """cos/sin tables matching reference._rope_tables numerics (f32 freqs)."""
    inv = (1.0 / (np.float32(ROPE_BASE) **
                  (np.arange(0, hd, 2, dtype=np.float32) / np.float32(hd))))
    inv = inv.astype(np.float32)
    freqs = (np.arange(S, dtype=np.float32)[:, None] * inv[None, :]
             ).astype(np.float32)
    cos = np.cos(freqs.astype(np.float64)).astype(np.float32)
    sin = np.sin(freqs.astype(np.float64)).astype(np.float32)
    return cos, sin


def _bf16():
    import ml_dtypes
    return ml_dtypes.bfloat16


def make_const_inputs(S):
    """Constant per-core inputs: packed RoPE tables + causal tri mask."""
    bf = _bf16()
    cos, sin = rope_tables(S, HEAD_DIM)
    cs = np.concatenate([cos, -sin, sin], axis=1).astype(np.float32)
    return {
        "cs": np.ascontiguousarray(cs),
        "tri": np.triu(np.ones((128, 128), dtype=np.float32)).astype(bf),
        "ident": np.eye(128, dtype=np.float32).astype(bf),
    }


def _is_causal_mask(mask, S):
    m = mask.reshape(S, S)
    rows = np.unique(np.concatenate([np.arange(0, S, max(S // 64, 1)),
                                     [S - 1]]))
    for r in rows:
        row = m[r]
        if not np.all(row[:r + 1] == 0.0):
            return False
        if r + 1 < S and not np.all(row[r + 1:] <= -50.0):
            return False
    return True


_NC_CACHE = {}


def kernel(hidden_states, attention_mask, Wqkv, Wo):
    B, S, H = hidden_states.shape
    nh, hd = NUM_HEADS, HEAD_DIM
    HL = nh // (N_CORES // B)       # heads per core
    DL = HL * hd
    G = N_CORES // B                # cores per batch

    if not _is_causal_mask(np.asarray(attention_mask), S):
        # general-mask fallback: exact host computation
        return _host_reference(hidden_states, attention_mask, Wqkv, Wo)

    key = (S, H, HL)
    if key not in _NC_CACHE:
        _NC_CACHE[key] = build_nc(S, H, HL)
    nc = _NC_CACHE[key]

    bf = _bf16()
    consts = make_const_inputs(S)

    hs = np.asarray(hidden_states, dtype=np.float32)
    Wqkv = np.asarray(Wqkv, dtype=np.float32)
    Wo = np.asarray(Wo, dtype=np.float32)
    hT = [np.ascontiguousarray(hs[b].T).astype(bf) for b in range(B)]

    in_maps = []
    for c in range(N_CORES):
        b, g = divmod(c, G)
        c0 = g * DL
        wqkv = np.concatenate([
            Wqkv[:, 2 * H + c0:2 * H + c0 + DL],   # V
            Wqkv[:, c0:c0 + DL],                   # Q
            Wqkv[:, H + c0:H + c0 + DL],           # K
        ], axis=1).astype(bf)
        in_maps.append({
            "hT": hT[b],
            "wqkv": np.ascontiguousarray(wqkv),
            "wo": np.ascontiguousarray(Wo[c0:c0 + DL, :].astype(bf)),
            **consts,
        })

    res = run_bass_kernel_spmd(nc, in_maps, list(range(N_CORES)))
    out = np.empty((B, S, H), dtype=np.float32)
    for b in range(B):
        acc = res.results[b * G]["part"].astype(np.float64)
        for g in range(1, G):
            acc += res.results[b * G + g]["part"]
        out[b] = acc.astype(np.float32)
    return out


def _host_reference(hidden_states, attention_mask, Wqkv, Wo):
    """Exact fallback for non-causal masks (numpy, fp32)."""
    B, S, H = hidden_states.shape
    nh, hd = NUM_HEADS, HEAD_DIM
    cos, sin = rope_tables(S, hd)
    qkv = hidden_states.reshape(B * S, H) @ Wqkv
    qkv = qkv.reshape(B, S, 3, nh, hd).transpose(2, 0, 3, 1, 4)
    q, k, v = qkv[0], qkv[1], qkv[2]

    def rope(x):
        x1, x2 = x[..., :hd // 2], x[..., hd // 2:]
        c, s = cos[None, None], sin[None, None]
        return np.concatenate([x1 * c - x2 * s, x2 * c + x1 * s], axis=-1)

    q, k = rope(q), rope(k)
    scores = np.einsum('bhqd,bhkd->bhqk', q, k) * (hd ** -0.5)
    scores = scores + attention_mask.reshape(1, 1, S, S)
    scores -= scores.max(axis=-1, keepdims=True)
    e = np.exp(scores)
    attn = e / e.sum(axis=-1, keepdims=True)
    out = np.einsum('bhqk,bhkd->bhqd', attn, v)
    out = out.transpose(0, 2, 1, 3).reshape(B, S, H)
    return (out @ Wo).astype(np.float32)


# revision 45
# speedup vs baseline: 1.2168x; 1.0211x over previous
"""Causal self-attention (RoPE) Trainium2 Bass kernel, SPMD over 8 NeuronCores.

Sharding: data-parallel over batch (B=2) x tensor-parallel over heads
(16 heads -> 4 heads per core).  core c handles batch c//4, heads
4*(c%4) .. 4*(c%4)+3.  Each core computes its heads' attention output and a
partial out@Wo contribution ([S, H]); the host sums the 4 partials per batch.

Device pipeline per core (q-tile-outer formulation, bf16 matmuls):
  phase1(st): QKV projection (bf16) -> RoPE on DVE/Pool -> x_sb bf16 ->
     tiled DMA-transpose into persistent qT/kT; V (bf16, with an appended
     ones column) kept natural in vo.
  phase2(j):  for each head: transposed score tiles for k-tiles t<=j are
     stacked in PSUM columns -> one exp per <=1024-col stack (ACT) ->
     causal tri mask on the diagonal tile (DVE) -> AV matmuls with the
     P^T tile as STATIONARY and [V | 1] as moving (J=65) accumulating
     O[q, d] + the softmax denominator (col 64) in PSUM.  Normalize via
     reciprocal + broadcast mul, DMA-transpose O^T for phase 3.
  phase3(st): out = O^T @ Wo partial, streamed behind phase 2.
The three phases stream st/j/st-lagged so PE never starves.
"""

import sys
import numpy as np

for _p in ("/opt/trn_rl_repo", "/root/.axon_site/_ro/trn_rl_repo"):
    if _p not in sys.path:
        sys.path.append(_p)

import concourse.bacc as bacc
from concourse import mybir
from concourse.tile import TileContext
from concourse.bass_utils import run_bass_kernel_spmd

F32 = mybir.dt.float32
BF16 = mybir.dt.bfloat16
EXP = mybir.ActivationFunctionType.Exp

NUM_HEADS = 16
HEAD_DIM = 64
ROPE_BASE = 160000.0
N_CORES = 8


def build_nc(S, H, HL, AVKEEP=2, LAG_=2, PH3POS='end'):
    """Build the SPMD Bass program.

    S: sequence length; H: hidden size; HL: heads per core (local).
    Per-core tensors: hT [H,S] bf16, wqkv [H, 3*DL] bf16 (cols V|Q|K),
    wo [DL,H] bf16, cs [S,96] f32 (cos|: -sin|: +sin), tri [128,128] bf16
    -> out part [S,H] f32.
    """
    DL = HL * HEAD_DIM          # local channels (256)
    NI = H // 128               # contraction tiles for projections (8)
    NS = S // 128               # sequence tiles (16)
    CT = DL // 128              # channel tiles (2)
    assert DL % 128 == 0 and H % 128 == 0 and S % 128 == 0
    scale = HEAD_DIM ** -0.5

    nc = bacc.Bacc("TRN2", target_bir_lowering=False, debug=False,
                   num_devices=N_CORES)
    hT_d = nc.declare_dram_parameter("hT", [H, S], BF16, isOutput=False)
    wqkv_d = nc.declare_dram_parameter("wqkv", [H, 3 * DL], BF16, isOutput=False)
    wo_d = nc.declare_dram_parameter("wo", [DL, H], BF16, isOutput=False)
    cs_d = nc.declare_dram_parameter("cs", [S, 96], F32, isOutput=False)
    tri_d = nc.declare_dram_parameter("tri", [128, 128], BF16, isOutput=False)
    id_d = nc.declare_dram_parameter("ident", [128, 128], BF16, isOutput=False)
    out_d = nc.declare_dram_parameter("part", [S, H], F32, isOutput=True)

    with TileContext(nc) as tc:
        with (
            tc.tile_pool(name="w", bufs=1) as w_pool,
            tc.tile_pool(name="persist", bufs=1) as pers,
            tc.tile_pool(name="hstream", bufs=5) as hs_pool,
            tc.tile_pool(name="rope", bufs=2) as rope_pool,
            tc.tile_pool(name="xsb", bufs=4) as x_pool,
            tc.tile_pool(name="psb", bufs=4) as p_pool,
            tc.tile_pool(name="norm", bufs=4) as n_pool,
            tc.tile_pool(name="osb", bufs=3) as o_pool,
            tc.tile_pool(name="ps_a", bufs=3, space="PSUM") as ps_a,
            tc.tile_pool(name="ps_o", bufs=2, space="PSUM") as ps_o,
        ):
            # --- weights / constants (resident) ---
            w_t = w_pool.tile([128, NI, 3 * DL], BF16)
            wo_t = w_pool.tile([128, CT, H], BF16)
            cs_t = w_pool.tile([128, NS, 96], F32)
            tri_t = w_pool.tile([128, 128], BF16)
            id_t = w_pool.tile([128, 128], BF16)
            # weights go down the ACT HWDGE queue (SP streams h tiles);
            # per-i-tile DMAs so the first projection matmul can start as
            # soon as its slice lands.
            wr = wqkv_d[:].rearrange("(t p) d -> p t d", p=128)
            for i in range(NI):
                nc.scalar.dma_start(out=w_t[:, i:i + 1, :], in_=wr[:, i:i + 1, :])
            nc.scalar.dma_start(
                out=wo_t, in_=wo_d[:].rearrange("(t p) o -> p t o", p=128))
            nc.scalar.dma_start(
                out=cs_t, in_=cs_d[:].rearrange("(t p) c -> p t c", p=128))
            nc.scalar.dma_start(out=tri_t, in_=tri_d[:])
            nc.scalar.dma_start(out=id_t, in_=id_d[:])

            qkT = pers.tile([128, 2, CT, S], BF16)   # [d, q/k, ct, s]
            vo = pers.tile([128, NS, HL, 65], BF16)  # [k, ktile, h, d|1]
            oT = pers.tile([128, CT, S], BF16)       # [dl, ct, s]
            nc.gpsimd.memset(vo[:, :, :, 64:65], 1.0)

            hT_r = hT_d[:].rearrange("(t p) s -> p t s", p=128)
            OV, OQ = 0, DL          # offsets in qkv psum: V | Q,K

            # DMA deferral: a HWDGE sequencer processes DMAs in order and
            # BLOCKS while a DMA's source isn't ready, so every DMA whose
            # source is produced in iteration i is dispatched in iteration
            # i+2 (source long done -> the queue never stalls).  Producer
            # DMAs (h loads, qkT) go down SP; consumer-side stores (oT,
            # out) go down the ACT queue, which is otherwise idle between
            # exps.
            spq, spq_next, actq, actq_next = [], [], [], []

            def flush_dma_queues():
                while spq:
                    spq.pop(0)()
                while actq:
                    actq.pop(0)()
                spq.extend(spq_next)
                actq.extend(actq_next)
                del spq_next[:], actq_next[:]

            # ---------------- phase 1 (split into chunks) ----------------
            def phase1_load(st):
                s0 = st * 128
                h_t = hs_pool.tile([128, NI, 128], BF16, tag="h")
                nc.sync.dma_start(out=h_t, in_=hT_r[:, :, s0:s0 + 128])
                flush_dma_queues()
                qkv_ps = ps_a.tile([128, 1024], F32, tag="psa")
                return h_t, qkv_ps

            def phase1_qkv(h_t, qkv_ps, group):
                lo, hi = (0, 512) if group == 0 else (512, 768)
                for i in range(NI):
                    nc.tensor.matmul(qkv_ps[:, lo:hi], h_t[:, i, :],
                                     w_t[:, i, lo:hi],
                                     start=(i == 0), stop=(i == NI - 1))

            def phase1_post(st, qkv_ps):
                s0 = st * 128
                # V copy first: AV matmuls need vo sooner than qkT's
                # consumers need the rope output.  DVE -- gpsimd cannot
                # read PSUM.
                nc.vector.tensor_copy(
                    vo[:, st, :, 0:64],
                    qkv_ps[:, OV:OV + DL].rearrange("p (h d) -> p h d", d=64))

                # RoPE directly on the PSUM qkv (cols OQ..OQ+2*DL = q|k)
                cosb = cs_t[:, st, 0:32].unsqueeze(1).broadcast_to(
                    [128, 4 * HL, 32])
                sinmb = cs_t[:, st, 32:64].unsqueeze(1).broadcast_to(
                    [128, 2 * HL, 32])
                sinpb = cs_t[:, st, 64:96].unsqueeze(1).broadcast_to(
                    [128, 2 * HL, 32])
                x_qk = qkv_ps[:, OQ:OQ + 2 * DL]
                x4 = x_qk.rearrange("p (r two d) -> p r two d", two=2, d=32)
                a_t = rope_pool.tile([128, 2 * DL], F32, tag="ra")
                nc.vector.tensor_mul(
                    a_t.rearrange("p (r d) -> p r d", d=32),
                    x_qk.rearrange("p (r d) -> p r d", d=32), cosb)
                b_t = rope_pool.tile([128, 2 * DL], F32, tag="rb")
                b4 = b_t.rearrange("p (r two d) -> p r two d", two=2, d=32)
                nc.vector.tensor_mul(b4[:, :, 0, :], x4[:, :, 1, :], sinmb)
                nc.vector.tensor_mul(b4[:, :, 1, :], x4[:, :, 0, :], sinpb)
                x_sb = x_pool.tile([128, 2 * DL], BF16, tag="x")
                nc.gpsimd.tensor_add(x_sb, a_t, b_t)

                # tiled transpose q|k into qkT (one DMA, 4 128-blocks)
                spq_next.append(
                    lambda x_sb=x_sb, s0=s0: nc.sync.dma_start_transpose(
                        qkT[:, :, :, s0:s0 + 128], x_sb))

            # ---------------- phase 2 ----------------
            # AV matmuls are deferred two sub-stacks behind scores/exp so
            # the PE never waits on the ACT exp: the engine executes its
            # stream in order, so an AV emitted right after its exp stalls
            # the PE for the exp's full latency.
            avq = []   # entries (j, closure)
            peq = []   # deferred O-transpose closures (PE + DVE copy)
            o_live = {}

            def flush_avq(keep=0):
                while len(avq) > keep:
                    avq.pop(0)[1]()

            def flush_avq_j(j):
                while avq and avq[0][0] <= j:
                    avq.pop(0)[1]()

            def flush_peq():
                while peq:
                    peq.pop(0)()

            def phase2_stacks(j):
                """Generator: emits one scores+exp+mask sub-stack (with
                deferred AV) per step, yielding between stacks so PE filler
                chunks can be wedged in."""
                flush_peq()
                o_ps = ps_o.tile([128, 512], F32, tag="o")
                o_live[j] = o_ps
                for h in range(HL):
                    base = (h % 2) * 64
                    ct = h // 2
                    qT_h = qkT[base:base + 64, 0, ct, :]
                    kT_h = qkT[base:base + 64, 1, ct, :]
                    oc = h * 65
                    # sub-stacks of <=8 k-tiles (1024 PSUM cols)
                    for t0 in range(0, j + 1, 8):
                        nt = min(8, j + 1 - t0)
                        sc = ps_a.tile([128, 1024], F32, tag="psa")
                        for k in range(nt):
                            t = t0 + k
                            nc.tensor.matmul(
                                sc[:, k * 128:(k + 1) * 128],
                                kT_h[:, t * 128:(t + 1) * 128],
                                qT_h[:, j * 128:(j + 1) * 128],
                                start=True, stop=True)
                        flush_avq(keep=AVKEEP)
                        p_sb = p_pool.tile([128, 1024], BF16, tag="p")
                        nc.scalar.activation(p_sb[:, 0:nt * 128],
                                             sc[:, 0:nt * 128], EXP,
                                             scale=scale)
                        if t0 <= j < t0 + nt:  # diagonal tile: causal mask
                            dk = (j - t0) * 128
                            nc.vector.tensor_mul(p_sb[:, dk:dk + 128],
                                                 p_sb[:, dk:dk + 128], tri_t)

                        def av(h=h, t0=t0, nt=nt, oc=oc, p_sb=p_sb):
                            for k in range(nt):
                                t = t0 + k
                                nc.tensor.matmul(
                                    o_ps[:, oc:oc + 65],
                                    p_sb[:, k * 128:(k + 1) * 128],
                                    vo[:, t, h, :],
                                    start=(h == 0 and t == 0),
                                    stop=(t == j), skip_group_check=True)
                        avq.append((j, av))
                        yield

            def emit_phase2_scores(j):
                for _ in phase2_stacks(j):
                    pass

            def emit_phase2_norm(j):
                flush_avq_j(j)
                o_ps = o_live.pop(j)
                # snapshot the raw accumulator to SBUF immediately so the
                # PSUM bank frees for j+2's scores; recip/norm run off the
                # snapshot, off the release path
                o_raw = n_pool.tile([128, HL * 65], F32, tag="oraw")
                nc.vector.tensor_copy(o_raw, o_ps[:, 0:HL * 65])
                ov = o_raw.rearrange("p (h d) -> p h d", d=65)
                r_sb = n_pool.tile([128, HL], F32, tag="r")
                nc.vector.reciprocal(r_sb, ov[:, :, 64])
                o_sb = n_pool.tile([128, DL], BF16, tag="on")
                nc.vector.tensor_mul(
                    o_sb.rearrange("p (h d) -> p h d", d=64),
                    ov[:, :, 0:64],
                    r_sb.unsqueeze(2).broadcast_to([128, HL, 64]))

                # O^T on the PE (bf16 transpose, 53ns/tile) into a scores
                # psum slot, then one DVE copy into oT.  Deferred one
                # iteration so the PE never waits on the norm.
                def otrans(j=j, o_sb=o_sb):
                    t_ps = ps_a.tile([128, 1024], F32, tag="psa")
                    tsl = t_ps[:, 0:128].bitcast(BF16)
                    for ct in range(CT):
                        nc.tensor.transpose(
                            tsl[:, ct * 128:(ct + 1) * 128],
                            o_sb[:, ct * 128:(ct + 1) * 128], id_t)
                    nc.vector.tensor_copy(
                        oT[:, :, j * 128:(j + 1) * 128],
                        tsl.rearrange("p (c s) -> p c s", s=128))
                peq.append(otrans)

            # ---------------- phase 3 ----------------
            def emit_phase3(st):
                s0 = st * 128
                wo_ps = ps_a.tile([128, 1024], F32, tag="psa")
                for oc in (0, 512):
                    for ct in range(CT):
                        nc.tensor.matmul(wo_ps[:, oc:oc + 512],
                                         oT[:, ct, s0:s0 + 128],
                                         wo_t[:, ct, oc:oc + 512],
                                         start=(ct == 0), stop=(ct == CT - 1))
                out_sb = o_pool.tile([128, H], F32, tag="out")
                nc.vector.tensor_copy(out_sb[:, 0:512], wo_ps[:, 0:512])
                nc.scalar.copy(out_sb[:, 512:1024], wo_ps[:, 512:1024])
                actq_next.append(lambda s0=s0, out_sb=out_sb: nc.scalar.dma_start(
                    out=out_d[s0:s0 + 128, :], in_=out_sb))

            # ---------------- orchestration ----------------
            # Each iteration interleaves one q-tile of phase 2 (ACT-heavy:
            # the exps outweigh the scores+AV matmuls) with the PE-heavy
            # chunks of phase 1/3 as filler between sub-stacks, norm lagged
            # one iteration so the trailing AVs flush behind fresh PE work.
            LAG = LAG_

            def emit_iteration(st, j):
                fillers = []
                if st is not None:
                    h_t, qkv_ps = phase1_load(st)
                    phase1_qkv(h_t, qkv_ps, 0)
                    phase1_qkv(h_t, qkv_ps, 1)
                    phase1_post(st, qkv_ps)
                else:
                    flush_dma_queues()
                if j is not None:
                    n = 0
                    for _ in phase2_stacks(j):
                        n += 1
                        if PH3POS == 'mid' and n == 2 and j - 2 >= 0:
                            emit_phase3(j - 2)
                    if PH3POS == 'end' and j - 2 >= 0:
                        emit_phase3(j - 2)
                    if j - 1 >= 0:
                        emit_phase2_norm(j - 1)

            for st in range(LAG):
                emit_iteration(st, None)
            for st in range(LAG, NS):
                emit_iteration(st, st - LAG)
            for j in range(NS - LAG, NS):
                emit_iteration(None, j)
            flush_peq()
            emit_phase3(NS - 2)
            emit_phase2_norm(NS - 1)
            flush_peq()
            emit_phase3(NS - 1)
            flush_dma_queues()
            flush_dma_queues()

    nc.finalize()
    return nc


def rope_tables(S, hd):
    """cos/sin tables matching reference._rope_tables numerics (f32 freqs)."""
    inv = (1.0 / (np.float32(ROPE_BASE) **
                  (np.arange(0, hd, 2, dtype=np.float32) / np.float32(hd))))
    inv = inv.astype(np.float32)
    freqs = (np.arange(S, dtype=np.float32)[:, None] * inv[None, :]
             ).astype(np.float32)
    cos = np.cos(freqs.astype(np.float64)).astype(np.float32)
    sin = np.sin(freqs.astype(np.float64)).astype(np.float32)
    return cos, sin


def _bf16():
    import ml_dtypes
    return ml_dtypes.bfloat16


def make_const_inputs(S):
    """Constant per-core inputs: packed RoPE tables + causal tri mask."""
    bf = _bf16()
    cos, sin = rope_tables(S, HEAD_DIM)
    cs = np.concatenate([cos, -sin, sin], axis=1).astype(np.float32)
    return {
        "cs": np.ascontiguousarray(cs),
        "tri": np.triu(np.ones((128, 128), dtype=np.float32)).astype(bf),
        "ident": np.eye(128, dtype=np.float32).astype(bf),
    }


def _is_causal_mask(mask, S):
    m = mask.reshape(S, S)
    rows = np.unique(np.concatenate([np.arange(0, S, max(S // 64, 1)),
                                     [S - 1]]))
    for r in rows:
        row = m[r]
        if not np.all(row[:r + 1] == 0.0):
            return False
        if r + 1 < S and not np.all(row[r + 1:] <= -50.0):
            return False
    return True


_NC_CACHE = {}


def kernel(hidden_states, attention_mask, Wqkv, Wo):
    B, S, H = hidden_states.shape
    nh, hd = NUM_HEADS, HEAD_DIM
    HL = nh // (N_CORES // B)       # heads per core
    DL = HL * hd
    G = N_CORES // B                # cores per batch

    if not _is_causal_mask(np.asarray(attention_mask), S):
        # general-mask fallback: exact host computation
        return _host_reference(hidden_states, attention_mask, Wqkv, Wo)

    key = (S, H, HL)
    if key not in _NC_CACHE:
        _NC_CACHE[key] = build_nc(S, H, HL)
    nc = _NC_CACHE[key]

    bf = _bf16()
    consts = make_const_inputs(S)

    hs = np.asarray(hidden_states, dtype=np.float32)
    Wqkv = np.asarray(Wqkv, dtype=np.float32)
    Wo = np.asarray(Wo, dtype=np.float32)
    hT = [np.ascontiguousarray(hs[b].T).astype(bf) for b in range(B)]

    in_maps = []
    for c in range(N_CORES):
        b, g = divmod(c, G)
        c0 = g * DL
        wqkv = np.concatenate([
            Wqkv[:, 2 * H + c0:2 * H + c0 + DL],   # V
            Wqkv[:, c0:c0 + DL],                   # Q
            Wqkv[:, H + c0:H + c0 + DL],           # K
        ], axis=1).astype(bf)
        in_maps.append({
            "hT": hT[b],
            "wqkv": np.ascontiguousarray(wqkv),
            "wo": np.ascontiguousarray(Wo[c0:c0 + DL, :].astype(bf)),
            **consts,
        })

    res = run_bass_kernel_spmd(nc, in_maps, list(range(N_CORES)))
    out = np.empty((B, S, H), dtype=np.float32)
    for b in range(B):
        acc = res.results[b * G]["part"].astype(np.float64)
        for g in range(1, G):
            acc += res.results[b * G + g]["part"]
        out[b] = acc.astype(np.float32)
    return out


def _host_reference(hidden_states, attention_mask, Wqkv, Wo):
    """Exact fallback for non-causal masks (numpy, fp32)."""
    B, S, H = hidden_states.shape
    nh, hd = NUM_HEADS, HEAD_DIM
    cos, sin = rope_tables(S, hd)
    qkv = hidden_states.reshape(B * S, H) @ Wqkv
    qkv = qkv.reshape(B, S, 3, nh, hd).transpose(2, 0, 3, 1, 4)
    q, k, v = qkv[0], qkv[1], qkv[2]

    def rope(x):
        x1, x2 = x[..., :hd // 2], x[..., hd // 2:]
        c, s = cos[None, None], sin[None, None]
        return np.concatenate([x1 * c - x2 * s, x2 * c + x1 * s], axis=-1)

    q, k = rope(q), rope(k)
    scores = np.einsum('bhqd,bhkd->bhqk', q, k) * (hd ** -0.5)
    scores = scores + attention_mask.reshape(1, 1, S, S)
    scores -= scores.max(axis=-1, keepdims=True)
    e = np.exp(scores)
    attn = e / e.sum(axis=-1, keepdims=True)
    out = np.einsum('bhqk,bhkd->bhqd', attn, v)
    out = out.transpose(0, 2, 1, 3).reshape(B, S, H)
    return (out @ Wo).astype(np.float32)


# revision 60
# speedup vs baseline: 1.2450x; 1.0231x over previous
"""Causal self-attention (RoPE) Trainium2 Bass kernel, SPMD over 8 NeuronCores.

Sharding: data-parallel over batch (B=2) x tensor-parallel over heads
(16 heads -> 4 heads per core).  core c handles batch c//4, heads
4*(c%4) .. 4*(c%4)+3.  Each core computes its heads' attention output and a
partial out@Wo contribution ([S, H]); the host sums the 4 partials per batch.

Device pipeline per core (q-tile-outer formulation, bf16 matmuls):
  phase1(st): QKV projection (bf16) -> RoPE on DVE/Pool -> x_sb bf16 ->
     tiled DMA-transpose into persistent qT/kT; V (bf16, with an appended
     ones column) kept natural in vo.
  phase2(j):  for each head: transposed score tiles for k-tiles t<=j are
     stacked in PSUM columns -> one exp per <=1024-col stack (ACT) ->
     causal tri mask on the diagonal tile (DVE) -> AV matmuls with the
     P^T tile as STATIONARY and [V | 1] as moving (J=65) accumulating
     O[q, d] + the softmax denominator (col 64) in PSUM.  Normalize via
     reciprocal + broadcast mul, DMA-transpose O^T for phase 3.
  phase3(st): out = O^T @ Wo partial, streamed behind phase 2.
The three phases stream st/j/st-lagged so PE never starves.
"""

import sys
import numpy as np

for _p in ("/opt/trn_rl_repo", "/root/.axon_site/_ro/trn_rl_repo"):
    if _p not in sys.path:
        sys.path.append(_p)

import concourse.bacc as bacc
from concourse import mybir
from concourse.tile import TileContext
from concourse.bass_utils import run_bass_kernel_spmd

F32 = mybir.dt.float32
BF16 = mybir.dt.bfloat16
EXP = mybir.ActivationFunctionType.Exp

NUM_HEADS = 16
HEAD_DIM = 64
ROPE_BASE = 160000.0
N_CORES = 8


def build_nc(S, H, HL, AVKEEP=2, LAG_=2, PH3POS='end'):
    """Build the SPMD Bass program.

    S: sequence length; H: hidden size; HL: heads per core (local).
    Per-core tensors: hT [H,S] bf16, wqkv [H, 3*DL] bf16 (cols V|Q|K),
    wo [DL,H] bf16, cs [S,96] f32 (cos|: -sin|: +sin), tri [128,128] bf16
    -> out part [S,H] f32.
    """
    DL = HL * HEAD_DIM          # local channels (256)
    NI = H // 128               # contraction tiles for projections (8)
    NS = S // 128               # sequence tiles (16)
    CT = DL // 128              # channel tiles (2)
    assert DL % 128 == 0 and H % 128 == 0 and S % 128 == 0
    scale = HEAD_DIM ** -0.5

    nc = bacc.Bacc("TRN2", target_bir_lowering=False, debug=False,
                   num_devices=N_CORES)
    hT_d = nc.declare_dram_parameter("hT", [H, S], BF16, isOutput=False)
    wqkv_d = nc.declare_dram_parameter("wqkv", [H, 3 * DL], BF16, isOutput=False)
    wo_d = nc.declare_dram_parameter("wo", [DL, H], BF16, isOutput=False)
    cs_d = nc.declare_dram_parameter("cs", [S, 96], BF16, isOutput=False)
    tri_d = nc.declare_dram_parameter("tri", [128, 128], BF16, isOutput=False)
    id_d = nc.declare_dram_parameter("ident", [128, 128], BF16, isOutput=False)
    out_d = nc.declare_dram_parameter("part", [S, H], F32, isOutput=True)

    with TileContext(nc) as tc:
        with (
            tc.tile_pool(name="w", bufs=1) as w_pool,
            tc.tile_pool(name="persist", bufs=1) as pers,
            tc.tile_pool(name="hstream", bufs=5) as hs_pool,
            tc.tile_pool(name="rope", bufs=3) as rope_pool,
            tc.tile_pool(name="xsb", bufs=4) as x_pool,
            tc.tile_pool(name="psb", bufs=6) as p_pool,
            tc.tile_pool(name="norm", bufs=4) as n_pool,
            tc.tile_pool(name="osb", bufs=3) as o_pool,
            tc.tile_pool(name="ps_a", bufs=3, space="PSUM") as ps_a,
            tc.tile_pool(name="ps_o", bufs=2, space="PSUM") as ps_o,
        ):
            # --- weights / constants (resident) ---
            w_t = w_pool.tile([128, NI, 3 * DL], BF16)
            wo_t = w_pool.tile([128, CT, H], BF16)
            cs_t = w_pool.tile([128, NS, 96], BF16)
            tri_t = w_pool.tile([128, 128], BF16)
            id_t = w_pool.tile([128, 128], BF16)
            # weights go down the ACT HWDGE queue (SP streams h tiles);
            # per-i-tile DMAs so the first projection matmul can start as
            # soon as its slice lands.
            wr = wqkv_d[:].rearrange("(t p) d -> p t d", p=128)
            for i0 in range(0, NI, 4):
                nc.scalar.dma_start(out=w_t[:, i0:i0 + 4, :],
                                    in_=wr[:, i0:i0 + 4, :])
            nc.scalar.dma_start(
                out=cs_t, in_=cs_d[:].rearrange("(t p) c -> p t c", p=128))
            nc.scalar.dma_start(out=tri_t, in_=tri_d[:])

            def late_consts():
                nc.scalar.dma_start(
                    out=wo_t, in_=wo_d[:].rearrange("(t p) o -> p t o", p=128))
                nc.scalar.dma_start(out=id_t, in_=id_d[:])

            qkT = pers.tile([128, 2, CT, S], BF16)   # [d, q/k, ct, s]
            vo = pers.tile([128, NS, HL, 65], BF16)  # [k, ktile, h, d|1]
            oT = pers.tile([128, CT, S], BF16)       # [dl, ct, s]
            nc.gpsimd.memset(vo[:, :, :, 64:65], 1.0)

            hT_r = hT_d[:].rearrange("(t p) s -> p t s", p=128)
            OV, OQ = 0, DL          # offsets in qkv psum: V | Q,K

            # DMA deferral: a HWDGE sequencer processes DMAs in order and
            # BLOCKS while a DMA's source isn't ready, so every DMA whose
            # source is produced in iteration i is dispatched in iteration
            # i+2 (source long done -> the queue never stalls).  Producer
            # DMAs (h loads, qkT) go down SP; consumer-side stores (oT,
            # out) go down the ACT queue, which is otherwise idle between
            # exps.
            spq, spq_next, actq, actq_next = [], [], [], []

            def flush_dma_queues():
                while spq:
                    spq.pop(0)()
                while actq:
                    actq.pop(0)()
                spq.extend(spq_next)
                actq.extend(actq_next)
                del spq_next[:], actq_next[:]

            # ---------------- phase 1 (split into chunks) ----------------
            def phase1_load(st):
                s0 = st * 128
                h_t = hs_pool.tile([128, NI, 128], BF16, tag="h")
                nc.sync.dma_start(out=h_t, in_=hT_r[:, :, s0:s0 + 128])
                flush_dma_queues()
                qkv_ps = ps_a.tile([128, 1024], F32, tag="psa")
                return h_t, qkv_ps

            def phase1_qkv(h_t, qkv_ps, group):
                lo, hi = (0, 512) if group == 0 else (512, 768)
                for i in range(NI):
                    nc.tensor.matmul(qkv_ps[:, lo:hi], h_t[:, i, :],
                                     w_t[:, i, lo:hi],
                                     start=(i == 0), stop=(i == NI - 1))

            def phase1_post(st, qkv_ps):
                s0 = st * 128
                # V copy first: AV matmuls need vo sooner than qkT's
                # consumers need the rope output.  DVE -- gpsimd cannot
                # read PSUM.
                nc.vector.tensor_copy(
                    vo[:, st, :, 0:64],
                    qkv_ps[:, OV:OV + DL].rearrange("p (h d) -> p h d", d=64))

                # RoPE directly on the PSUM qkv (cols OQ..OQ+2*DL = q|k)
                cosb = cs_t[:, st, 0:32].unsqueeze(1).broadcast_to(
                    [128, 4 * HL, 32])
                sinmb = cs_t[:, st, 32:64].unsqueeze(1).broadcast_to(
                    [128, 2 * HL, 32])
                sinpb = cs_t[:, st, 64:96].unsqueeze(1).broadcast_to(
                    [128, 2 * HL, 32])
                x_qk = qkv_ps[:, OQ:OQ + 2 * DL]
                x4 = x_qk.rearrange("p (r two d) -> p r two d", two=2, d=32)
                a_t = rope_pool.tile([128, 2 * DL], F32, tag="ra")
                nc.vector.tensor_mul(
                    a_t.rearrange("p (r d) -> p r d", d=32),
                    x_qk.rearrange("p (r d) -> p r d", d=32), cosb)
                b_t = rope_pool.tile([128, 2 * DL], F32, tag="rb")
                b4 = b_t.rearrange("p (r two d) -> p r two d", two=2, d=32)
                nc.vector.tensor_mul(b4[:, :, 0, :], x4[:, :, 1, :], sinmb)
                nc.vector.tensor_mul(b4[:, :, 1, :], x4[:, :, 0, :], sinpb)
                x_sb = x_pool.tile([128, 2 * DL], BF16, tag="x")
                nc.gpsimd.tensor_add(x_sb, a_t, b_t)

                # tiled transpose q|k into qkT (one DMA, 4 128-blocks).
                # First tiles use 1-iteration deferral: at startup the SP
                # queue is short so the earlier dispatch wins latency.
                q = spq if st < 3 else spq_next
                q.append(
                    lambda x_sb=x_sb, s0=s0: nc.sync.dma_start_transpose(
                        qkT[:, :, :, s0:s0 + 128], x_sb))

            # ---------------- phase 2 ----------------
            # AV matmuls are deferred two sub-stacks behind scores/exp so
            # the PE never waits on the ACT exp: the engine executes its
            # stream in order, so an AV emitted right after its exp stalls
            # the PE for the exp's full latency.
            avq = []   # entries (j, closure)
            peq = []   # deferred O-transpose closures (PE + DVE copy)
            o_live = {}

            def flush_avq(keep=0):
                while len(avq) > keep:
                    avq.pop(0)[1]()

            def flush_avq_j(j):
                while avq and avq[0][0] <= j:
                    avq.pop(0)[1]()

            def flush_peq():
                while peq:
                    peq.pop(0)()

            def phase2_stacks(j):
                """Generator: emits one scores+exp+mask sub-stack (with
                deferred AV) per step, yielding between stacks so PE filler
                chunks can be wedged in."""
                flush_peq()
                o_ps = ps_o.tile([128, 512], F32, tag="o")
                o_live[j] = o_ps
                for h in range(HL):
                    base = (h % 2) * 64
                    ct = h // 2
                    qT_h = qkT[base:base + 64, 0, ct, :]
                    kT_h = qkT[base:base + 64, 1, ct, :]
                    oc = h * 65
                    # sub-stacks of <=8 k-tiles (1024 PSUM cols)
                    for t0 in range(0, j + 1, 8):
                        nt = min(8, j + 1 - t0)
                        sc = ps_a.tile([128, 1024], F32, tag="psa")
                        for k in range(nt):
                            t = t0 + k
                            nc.tensor.matmul(
                                sc[:, k * 128:(k + 1) * 128],
                                kT_h[:, t * 128:(t + 1) * 128],
                                qT_h[:, j * 128:(j + 1) * 128],
                                start=True, stop=True)
                        flush_avq(keep=AVKEEP)
                        p_sb = p_pool.tile([128, 1024], BF16, tag="p")
                        nc.scalar.activation(p_sb[:, 0:nt * 128],
                                             sc[:, 0:nt * 128], EXP,
                                             scale=scale)
                        if t0 <= j < t0 + nt:  # diagonal tile: causal mask
                            dk = (j - t0) * 128
                            nc.vector.tensor_mul(p_sb[:, dk:dk + 128],
                                                 p_sb[:, dk:dk + 128], tri_t)

                        def av(h=h, t0=t0, nt=nt, oc=oc, p_sb=p_sb):
                            for k in range(nt):
                                t = t0 + k
                                nc.tensor.matmul(
                                    o_ps[:, oc:oc + 65],
                                    p_sb[:, k * 128:(k + 1) * 128],
                                    vo[:, t, h, :],
                                    start=(h == 0 and t == 0),
                                    stop=(t == j), skip_group_check=True)
                        avq.append((j, av))
                        yield

            def emit_phase2_scores(j):
                for _ in phase2_stacks(j):
                    pass

            def emit_phase2_norm(j):
                flush_avq_j(j)
                o_ps = o_live.pop(j)
                # snapshot the raw accumulator to SBUF immediately so the
                # PSUM bank frees for j+2's scores; recip/norm run off the
                # snapshot, off the release path
                o_raw = n_pool.tile([128, HL * 65], F32, tag="oraw")
                nc.vector.tensor_copy(o_raw, o_ps[:, 0:HL * 65])
                ov = o_raw.rearrange("p (h d) -> p h d", d=65)
                r_sb = n_pool.tile([128, HL], F32, tag="r")
                nc.vector.reciprocal(r_sb, ov[:, :, 64])
                o_sb = n_pool.tile([128, DL], BF16, tag="on")
                nc.vector.tensor_mul(
                    o_sb.rearrange("p (h d) -> p h d", d=64),
                    ov[:, :, 0:64],
                    r_sb.unsqueeze(2).broadcast_to([128, HL, 64]))

                # O^T on the PE (bf16 transpose, 53ns/tile) into a scores
                # psum slot, then one DVE copy into oT.  Deferred one
                # iteration so the PE never waits on the norm.
                def otrans(j=j, o_sb=o_sb):
                    t_ps = ps_a.tile([128, 1024], F32, tag="psa")
                    tsl = t_ps[:, 0:128].bitcast(BF16)
                    for ct in range(CT):
                        nc.tensor.transpose(
                            tsl[:, ct * 128:(ct + 1) * 128],
                            o_sb[:, ct * 128:(ct + 1) * 128], id_t)
                    nc.vector.tensor_copy(
                        oT[:, :, j * 128:(j + 1) * 128],
                        tsl.rearrange("p (c s) -> p c s", s=128))
                peq.append(otrans)

            # ---------------- phase 3 ----------------
            tail_mode = [False]

            def emit_phase3(st):
                s0 = st * 128
                wo_ps = ps_a.tile([128, 1024], F32, tag="psa")
                for oc in (0, 512):
                    for ct in range(CT):
                        nc.tensor.matmul(wo_ps[:, oc:oc + 512],
                                         oT[:, ct, s0:s0 + 128],
                                         wo_t[:, ct, oc:oc + 512],
                                         start=(ct == 0), stop=(ct == CT - 1))
                out_sb = o_pool.tile([128, H], F32, tag="out")
                nc.vector.tensor_copy(out_sb[:, 0:512], wo_ps[:, 0:512])
                nc.scalar.copy(out_sb[:, 512:1024], wo_ps[:, 512:1024])
                actq_next.append(lambda s0=s0, out_sb=out_sb: nc.scalar.dma_start(
                    out=out_d[s0:s0 + 128, :], in_=out_sb))

            # ---------------- orchestration ----------------
            # Each iteration interleaves one q-tile of phase 2 (ACT-heavy:
            # the exps outweigh the scores+AV matmuls) with the PE-heavy
            # chunks of phase 1/3 as filler between sub-stacks, norm lagged
            # one iteration so the trailing AVs flush behind fresh PE work.
            LAG = LAG_

            def emit_iteration(st, j):
                fillers = []
                if st is not None:
                    h_t, qkv_ps = phase1_load(st)
                    phase1_qkv(h_t, qkv_ps, 0)
                    phase1_qkv(h_t, qkv_ps, 1)
                    phase1_post(st, qkv_ps)
                else:
                    flush_dma_queues()
                if j is not None:
                    n = 0
                    for _ in phase2_stacks(j):
                        n += 1
                        if PH3POS == 'mid' and n == 2 and j - 2 >= 0:
                            emit_phase3(j - 2)
                    if PH3POS == 'end' and j - 2 >= 0:
                        emit_phase3(j - 2)
                    if j - 1 >= 0:
                        emit_phase2_norm(j - 1)

            for st in range(LAG):
                emit_iteration(st, None)
                if st == 0:
                    actq_next.append(late_consts)
            for st in range(LAG, NS):
                emit_iteration(st, st - LAG)
            tail_mode[0] = True
            for j in range(NS - LAG, NS):
                emit_iteration(None, j)
            flush_peq()
            emit_phase3(NS - 2)
            emit_phase2_norm(NS - 1)
            flush_peq()
            emit_phase3(NS - 1)
            flush_dma_queues()
            flush_dma_queues()

    nc.finalize()
    return nc


def rope_tables(S, hd):
    """cos/sin tables matching reference._rope_tables numerics (f32 freqs)."""
    inv = (1.0 / (np.float32(ROPE_BASE) **
                  (np.arange(0, hd, 2, dtype=np.float32) / np.float32(hd))))
    inv = inv.astype(np.float32)
    freqs = (np.arange(S, dtype=np.float32)[:, None] * inv[None, :]
             ).astype(np.float32)
    cos = np.cos(freqs.astype(np.float64)).astype(np.float32)
    sin = np.sin(freqs.astype(np.float64)).astype(np.float32)
    return cos, sin


def _bf16():
    import ml_dtypes
    return ml_dtypes.bfloat16


def make_const_inputs(S):
    """Constant per-core inputs: packed RoPE tables + causal tri mask."""
    bf = _bf16()
    cos, sin = rope_tables(S, HEAD_DIM)
    cs = np.concatenate([cos, -sin, sin], axis=1).astype(np.float32)
    return {
        "cs": np.ascontiguousarray(cs).astype(bf),
        "tri": np.triu(np.ones((128, 128), dtype=np.float32)).astype(bf),
        "ident": np.eye(128, dtype=np.float32).astype(bf),
    }


def _is_causal_mask(mask, S):
    m = mask.reshape(S, S)
    rows = np.unique(np.concatenate([np.arange(0, S, max(S // 64, 1)),
                                     [S - 1]]))
    for r in rows:
        row = m[r]
        if not np.all(row[:r + 1] == 0.0):
            return False
        if r + 1 < S and not np.all(row[r + 1:] <= -50.0):
            return False
    return True


_NC_CACHE = {}


def kernel(hidden_states, attention_mask, Wqkv, Wo):
    B, S, H = hidden_states.shape
    nh, hd = NUM_HEADS, HEAD_DIM
    HL = nh // (N_CORES // B)       # heads per core
    DL = HL * hd
    G = N_CORES // B                # cores per batch

    if not _is_causal_mask(np.asarray(attention_mask), S):
        # general-mask fallback: exact host computation
        return _host_reference(hidden_states, attention_mask, Wqkv, Wo)

    key = (S, H, HL)
    if key not in _NC_CACHE:
        _NC_CACHE[key] = build_nc(S, H, HL)
    nc = _NC_CACHE[key]

    bf = _bf16()
    consts = make_const_inputs(S)

    hs = np.asarray(hidden_states, dtype=np.float32)
    Wqkv = np.asarray(Wqkv, dtype=np.float32)
    Wo = np.asarray(Wo, dtype=np.float32)
    hT = [np.ascontiguousarray(hs[b].T).astype(bf) for b in range(B)]

    in_maps = []
    for c in range(N_CORES):
        b, g = divmod(c, G)
        c0 = g * DL
        wqkv = np.concatenate([
            Wqkv[:, 2 * H + c0:2 * H + c0 + DL],   # V
            Wqkv[:, c0:c0 + DL],                   # Q
            Wqkv[:, H + c0:H + c0 + DL],           # K
        ], axis=1).astype(bf)
        in_maps.append({
            "hT": hT[b],
            "wqkv": np.ascontiguousarray(wqkv),
            "wo": np.ascontiguousarray(Wo[c0:c0 + DL, :].astype(bf)),
            **consts,
        })

    res = run_bass_kernel_spmd(nc, in_maps, list(range(N_CORES)))
    out = np.empty((B, S, H), dtype=np.float32)
    for b in range(B):
        acc = res.results[b * G]["part"].astype(np.float64)
        for g in range(1, G):
            acc += res.results[b * G + g]["part"]
        out[b] = acc.astype(np.float32)
    return out


def _host_reference(hidden_states, attention_mask, Wqkv, Wo):
    """Exact fallback for non-causal masks (numpy, fp32)."""
    B, S, H = hidden_states.shape
    nh, hd = NUM_HEADS, HEAD_DIM
    cos, sin = rope_tables(S, hd)
    qkv = hidden_states.reshape(B * S, H) @ Wqkv
    qkv = qkv.reshape(B, S, 3, nh, hd).transpose(2, 0, 3, 1, 4)
    q, k, v = qkv[0], qkv[1], qkv[2]

    def rope(x):
        x1, x2 = x[..., :hd // 2], x[..., hd // 2:]
        c, s = cos[None, None], sin[None, None]
        return np.concatenate([x1 * c - x2 * s, x2 * c + x1 * s], axis=-1)

    q, k = rope(q), rope(k)
    scores = np.einsum('bhqd,bhkd->bhqk', q, k) * (hd ** -0.5)
    scores = scores + attention_mask.reshape(1, 1, S, S)
    scores -= scores.max(axis=-1, keepdims=True)
    e = np.exp(scores)
    attn = e / e.sum(axis=-1, keepdims=True)
    out = np.einsum('bhqk,bhkd->bhqd', attn, v)
    out = out.transpose(0, 2, 1, 3).reshape(B, S, H)
    return (out @ Wo).astype(np.float32)
